# revision 3
# baseline (speedup 1.0000x reference)
"""Block-diagonal matmul (BlockLinear) on 8 Trainium2 NeuronCores.

Problem: W [16, 64, 64] f32 stacked square blocks; inp [1024, 32768] f32.
out = block_diag(W) @ inp, i.e. per-block out[h] = W[h] @ inp[h*64:(h+1)*64, :].

Strategy (data parallel, per sharding hint):
  - Shard inp / out along the batch axis B=32768 across 8 cores (4096 cols each).
  - Host-side, pack the 16 64x64 blocks into 8 block-diagonal 128x128 pairs,
    pre-transposed for the TensorE "lhsT" stationary operand.
  - Per core: for each of the 8 row-pairs, DMA a [128, 4096] f32 slab in
    (2 MiB, HWDGE via sync engine), run 8 matmuls of N=512 into PSUM banks,
    copy PSUM->SBUF on VectorE, DMA the [128, 4096] result out (scalar engine
    HWDGE ring so stores don't serialize behind loads).

Memory-bound: 32.25 MiB HBM traffic per core ~= 94 us at ~358 GB/s.
"""

import os
import sys

import numpy as np

for _p in ("/opt/trn_rl_repo", "/opt/pypackages"):
    if os.path.isdir(_p) and _p not in sys.path:
        sys.path.append(_p)

H, D_BLK = 16, 64
D_TOTAL = H * D_BLK            # 1024
B = 32768
N_CORES = 8
BS = B // N_CORES              # 4096 batch columns per core
N_PAIR = H // 2                # 8 pairs of blocks -> 128 partitions each
FREE = 512                     # one PSUM bank of f32
NT = BS // FREE                # 8 matmuls per pair

_PROG_CACHE = {}


def _build_program(repeat: int = 1):
    import concourse.bacc as bacc
    import concourse.tile as tile
    from concourse import mybir

    f32 = mybir.dt.float32
    nc = bacc.Bacc("TRN2", target_bir_lowering=False, debug=False,
                   num_devices=N_CORES)

    w_d = nc.dram_tensor("w", (128, N_PAIR * 128), f32, kind="ExternalInput")
    x_d = nc.dram_tensor("x", (N_PAIR, 128, BS), f32, kind="ExternalInput")
    y_d = nc.dram_tensor("y", (N_PAIR, 128, BS), f32, kind="ExternalOutput")

    with tile.TileContext(nc) as tc:
        with (
            tc.tile_pool(name="wpool", bufs=1) as wpool,
            tc.tile_pool(name="xpool", bufs=3) as xpool,
            tc.tile_pool(name="ypool", bufs=3) as ypool,
            tc.tile_pool(name="psum", bufs=8, space="PSUM") as psum_pool,
        ):
            wt = wpool.tile([128, N_PAIR * 128], f32)
            nc.sync.dma_start(wt[:], w_d[:])

            def body():
                for p in range(N_PAIR):
                    xt = xpool.tile([128, BS], f32)
                    nc.sync.dma_start(xt[:], x_d[p])
                    yt = ypool.tile([128, BS], f32)
                    for n in range(NT):
                        ps = psum_pool.tile([128, FREE], f32)
                        nc.tensor.matmul(
                            ps[:],
                            wt[:, p * 128:(p + 1) * 128],
                            xt[:, n * FREE:(n + 1) * FREE],
                            start=True, stop=True,
                        )
                        nc.vector.tensor_copy(yt[:, n * FREE:(n + 1) * FREE], ps[:])
                    nc.scalar.dma_start(y_d[p], yt[:])

            if repeat == 1:
                body()
            else:
                with tc.For_i(0, repeat, 1):
                    body()

    nc.compile()
    return nc


def _get_program(repeat: int = 1):
    key = ("nc", repeat)
    if key not in _PROG_CACHE:
        _PROG_CACHE[key] = _build_program(repeat)
    return _PROG_CACHE[key]


def _pack_weights(W: np.ndarray) -> np.ndarray:
    """[16, 64, 64] -> [128, 8*128] lhsT layout: col p*128+m, row k holds
    block_diag(W[2p].T, W[2p+1].T)[k, m]."""
    WD = np.zeros((N_PAIR, 128, 128), dtype=np.float32)
    for p in range(N_PAIR):
        WD[p, :D_BLK, :D_BLK] = W[2 * p].T
        WD[p, D_BLK:, D_BLK:] = W[2 * p + 1].T
    return np.ascontiguousarray(WD.transpose(1, 0, 2).reshape(128, N_PAIR * 128))


def kernel(W: np.ndarray, inp: np.ndarray) -> np.ndarray:
    from concourse.bass_utils import run_bass_kernel_spmd

    W = np.asarray(W, dtype=np.float32)
    inp = np.asarray(inp, dtype=np.float32)
    assert W.shape == (H, D_BLK, D_BLK) and inp.shape == (D_TOTAL, B)

    nc = _get_program()
    w_host = _pack_weights(W)

    in_maps = []
    for c in range(N_CORES):
        x_shard = np.ascontiguousarray(inp[:, c * BS:(c + 1) * BS])
        in_maps.append({"w": w_host, "x": x_shard.reshape(N_PAIR, 128, BS)})

    res = run_bass_kernel_spmd(nc, in_maps, core_ids=list(range(N_CORES)))

    out = np.empty((D_TOTAL, B), dtype=np.float32)
    for c in range(N_CORES):
        out[:, c * BS:(c + 1) * BS] = res.results[c]["y"].reshape(D_TOTAL, BS)
    return out


if __name__ == "__main__":
    rng = np.random.default_rng(0)
    W = rng.standard_normal((H, D_BLK, D_BLK), dtype=np.float32)
    inp = rng.standard_normal((D_TOTAL, B), dtype=np.float32)
    out = kernel(W, inp)
    ref = np.einsum("hij,hjb->hib", W, inp.reshape(H, D_BLK, B)).reshape(D_TOTAL, B)
    err = np.abs(out - ref).max() / max(np.abs(ref).max(), 1e-9)
    print("self-check rel err:", err)
